# revision 16
# baseline (speedup 1.0000x reference)
"""Trainium2 Bass kernel for batched differentiable-Markowitz layer.

Solves, for each of 2048 rows p:  min_w 0.5 w'Sigma w + p'w  s.t. w in simplex,
matching a 200-step FISTA reference.  The fixed point is independent of lr and
the momentum schedule; we run 10 accelerated steps (8 bf16 + 2 float32r) with
constant momentum beta=0.5 and a serial loop engineered to be short:

  * momentum folded into the matmul: constant matrices A15 = 1.5(I - lr*Sigma)
    and A05n = -0.5(I - lr*Sigma) are pre-built, and PSUM accumulates
    z = wn_t@A15 + wn_{t-1}@A05n - lr*p in one group (the wn_{t-1} matmuls and
    the -lr*p identity-matmul run off the critical path since their operands
    exist a step early).
  * the projection threshold theta is LAGGED (updated on GpSimd off the
    critical path) and the iterate is re-normalized to the simplex sum every
    step (w_n = w/sum(w)), which stabilizes the lagged-theta iteration.  The
    whole on-path elementwise block runs in-order on DVE with no cross-engine
    hops: w = relu(z + th) with free row-sum accumulator, ic = 1/sv,
    wn = w*ic.
  * wn is transposed on the PE into the next step's stationary operand; the
    transpose/copy is split in 128-column halves so the k=0 matmul starts
    after half the copy.  Active-set count for theta's Newton step is
    refreshed every 4th step on GpSimd.
  * lr comes from a 4-iter on-device power iteration with a 1.10 safety
    factor.

Two batch chains of 128 rows run software-skewed (chain 1 one step behind).

Sharding: data-parallel over the batch, 256 rows per core, Sigma replicated,
no collectives.
"""

from contextlib import ExitStack

import numpy as np

import concourse.bass as bass  # noqa: F401
import concourse.tile as tile
from concourse import bacc, mybir
from concourse.bass_utils import run_bass_kernel_spmd

F32 = mybir.dt.float32
F32R = mybir.dt.float32r
BF16 = mybir.dt.bfloat16
OP = mybir.AluOpType
RELU = mybir.ActivationFunctionType.Relu

N = 256           # problem dimension
B_CORE = 256      # batch rows per core
N_CORES = 8
NB = B_CORE // 128
NK = N // 128

N_BF = 8          # bf16 matmul steps
N_R = 2           # float32r tail steps
K0_NEWTON = 2     # cold-start Newton iterations (step 0)
POW_ITERS = 4
L_SAFETY = 1.10
CNT_EVERY = 4     # refresh lagged 1/cnt every k-th step


def _make_identity(nc, ap, fill=1.0, base=0):
    nc.gpsimd.memset(ap, 0.0)
    nc.gpsimd.affine_select(
        out=ap, in_=ap, compare_op=OP.not_equal, fill=fill, base=base,
        pattern=[[-1, ap.shape[1]]], channel_multiplier=1)


def markowitz_tile_kernel(tc, out_w, in_p, in_sig, *,
                          n_bf=N_BF, n_r=N_R, k0=K0_NEWTON,
                          pow_iters=POW_ITERS, safety=L_SAFETY):
    nc = tc.nc
    ctx = ExitStack()
    n_steps = n_bf + n_r

    def mdt(t):          # matmul dtype of the iterate streamed at step t
        return BF16 if t < n_bf else F32R

    def edt(t):          # elementwise dtype of w at step t
        return BF16 if t < n_bf - 1 else F32

    const = ctx.enter_context(tc.tile_pool(name="const", bufs=1))
    rpool = ctx.enter_context(tc.tile_pool(name="r", bufs=4))
    wpool = ctx.enter_context(tc.tile_pool(name="w", bufs=4))
    ypool = ctx.enter_context(tc.tile_pool(name="y", bufs=4))
    wtpool = ctx.enter_context(tc.tile_pool(name="wt", bufs=6))
    mpool = ctx.enter_context(tc.tile_pool(name="m", bufs=2))
    xtpool = ctx.enter_context(tc.tile_pool(name="xt", bufs=4))
    ps_w = ctx.enter_context(tc.tile_pool(name="psw", bufs=3, space="PSUM"))
    ps_t = ctx.enter_context(tc.tile_pool(name="pst", bufs=2, space="PSUM"))
    ps_m = ctx.enter_context(tc.tile_pool(name="psm", bufs=2, space="PSUM"))

    with ctx:
        # ---- persistent state ----
        S = [const.tile([128, N], F32, name=f"S{k}") for k in range(NK)]
        P = [const.tile([128, N], F32, name=f"P{b}") for b in range(NB)]
        A15b = [const.tile([128, N], BF16, name=f"a15b{k}") for k in range(NK)]
        A05b = [const.tile([128, N], BF16, name=f"a05b{k}") for k in range(NK)]
        A15r = [const.tile([128, N], F32R, name=f"a15r{k}") for k in range(NK)]
        A05r = [const.tile([128, N], F32R, name=f"a05r{k}") for k in range(NK)]
        C_b = [const.tile([128, N], BF16, name=f"Cb{b}") for b in range(NB)]
        C_r = [const.tile([128, N], F32R, name=f"Cr{b}") for b in range(NB)]
        IA15 = [const.tile([128, N], F32, name=f"IA15{k}") for k in range(NK)]
        IA05 = [const.tile([128, N], F32, name=f"IA05{k}") for k in range(NK)]
        ID = const.tile([128, 128], F32, name="ID")
        ID_r = const.tile([128, 128], F32R, name="IDr")
        ID_b = const.tile([128, 128], BF16, name="IDb")
        ONES = const.tile([128, 1], F32, name="ONES")
        ZB = const.tile([128, N], BF16, name="ZB")
        Z1 = const.tile([128, 1], F32, name="Z1")
        w0T = const.tile([128, N], BF16, name="w0T")
        th = [const.tile([128, 1], F32, name=f"th{b}")[:] for b in range(NB)]
        sv = [const.tile([128, 1], F32, name=f"sv{b}")[:] for b in range(NB)]
        cv = [const.tile([128, 1], F32, name=f"cv{b}")[:] for b in range(NB)]
        cc = [const.tile([128, 1], F32, name=f"cc{b}")[:] for b in range(NB)]
        icn = [const.tile([128, 1], F32, name=f"icn{b}")[:] for b in range(NB)]
        ic = [const.tile([128, 1], F32, name=f"ic{b}")[:] for b in range(NB)]
        dl = [const.tile([128, 1], F32, name=f"dl{b}")[:] for b in range(NB)]
        lr_vec = const.tile([128, 1], F32, name="lrv")
        nlr_vec = const.tile([128, 1], F32, name="nlrv")
        nlr15 = const.tile([128, 1], F32, name="nlr15")
        lr05 = const.tile([128, 1], F32, name="lr05")
        ray = const.tile([1, 64], F32, name="ray")
        ray_i = const.tile([1, 64], F32, name="rayi")
        lmax = const.tile([1, 1], F32, name="lmax")
        lsafe = const.tile([1, 1], F32, name="lsafe")
        lr_s = const.tile([1, 1], F32, name="lrs")

        # ---- load inputs ----
        for k in range(NK):
            nc.sync.dma_start(S[k][:], in_sig[128 * k:128 * (k + 1), :])
        for b in range(NB):
            nc.sync.dma_start(P[b][:], in_p[128 * b:128 * (b + 1), :])

        # ---- constants ----
        _make_identity(nc, ID[:])
        nc.vector.tensor_copy(ID_r[:], ID[:])
        nc.vector.tensor_copy(ID_b[:], ID[:])
        for k in range(NK):
            _make_identity(nc, IA15[k][:], fill=1.5, base=128 * k)
            _make_identity(nc, IA05[k][:], fill=-0.5, base=128 * k)
        nc.gpsimd.memset(ONES[:], 1.0)
        nc.gpsimd.memset(ZB[:], 0.0)
        nc.gpsimd.memset(Z1[:], 0.0)
        nc.gpsimd.memset(w0T[:], 1.0 / N)

        # ---- power iteration for L (bf16, transposed layout) ----
        S_b = [const.tile([128, N], BF16, name=f"Sb{k}") for k in range(NK)]
        for k in range(NK):
            nc.vector.tensor_copy(S_b[k][:], S[k][:])
        xc = [S_b[k][:, 0:64] for k in range(NK)]
        xp = None
        for it in range(pow_iters):
            xn = []
            for j in range(NK):
                px = ps_m.tile([128, 64], F32, tag="pps", name="pps")
                for k in range(NK):
                    nc.tensor.matmul(px[:], S_b[k][:, 128 * j:128 * (j + 1)],
                                     xc[k],
                                     start=(k == 0), stop=(k == NK - 1))
                xs = xtpool.tile([128, 64], BF16, tag="xs", name="xs")
                nc.scalar.copy(xs[:], px[:])
                xn.append(xs)
            xp, xc = xc, [t[:] for t in xn]
        pnum = ps_m.tile([1, 64], F32, tag="pps", name="pps")
        pden = ps_m.tile([1, 64], F32, tag="pps", name="pps")
        for k in range(NK):
            prod_n = xtpool.tile([128, 64], F32, tag="prodn", name="prodn")
            prod_d = xtpool.tile([128, 64], F32, tag="prodd", name="prodd")
            nc.vector.tensor_tensor(prod_n[:], xc[k], xc[k], OP.mult)
            nc.vector.tensor_tensor(prod_d[:], xp[k], xc[k], OP.mult)
            nc.tensor.matmul(pnum[:], ONES[:], prod_n[:],
                             start=(k == 0), stop=(k == NK - 1))
            nc.tensor.matmul(pden[:], ONES[:], prod_d[:],
                             start=(k == 0), stop=(k == NK - 1))
        nc.vector.reciprocal(ray_i[:], pden[:])
        nc.vector.tensor_tensor(ray[:], pnum[:], ray_i[:], OP.mult)
        nc.vector.tensor_reduce(lmax[:], ray[:], axis=mybir.AxisListType.X, op=OP.max)
        nc.vector.tensor_scalar(lsafe[:], lmax[:], float(safety), None, OP.mult)
        nc.vector.reciprocal(lr_s[:], lsafe[:])
        nc.gpsimd.partition_broadcast(lr_vec[:], lr_s[:])
        nc.vector.tensor_scalar(nlr_vec[:], lr_vec[:], -1.0, None, OP.mult)
        nc.vector.tensor_scalar(nlr15[:], lr_vec[:], -1.5, None, OP.mult)
        nc.vector.tensor_scalar(lr05[:], lr_vec[:], 0.5, None, OP.mult)

        # ---- A15 = 1.5I - 1.5lr*S ; A05n = -0.5I + 0.5lr*S ; C = -lr*p ----
        for k in range(NK):
            nc.vector.scalar_tensor_tensor(A15b[k][:], S[k][:], nlr15[:, 0:1],
                                           IA15[k][:], op0=OP.mult, op1=OP.add)
            nc.vector.scalar_tensor_tensor(A05b[k][:], S[k][:], lr05[:, 0:1],
                                           IA05[k][:], op0=OP.mult, op1=OP.add)

        def emit_tail_builds():
            # f32r tail matrices; emitted after cold start so the DVE work
            # fills gaps during the first bf16 rounds.
            for k in range(NK):
                nc.vector.scalar_tensor_tensor(A15r[k][:], S[k][:],
                                               nlr15[:, 0:1], IA15[k][:],
                                               op0=OP.mult, op1=OP.add)
                nc.vector.scalar_tensor_tensor(A05r[k][:], S[k][:],
                                               lr05[:, 0:1], IA05[k][:],
                                               op0=OP.mult, op1=OP.add)
        for b in range(NB):
            nc.vector.tensor_scalar(C_b[b][:], P[b][:], nlr_vec[:, 0:1], None,
                                    OP.mult)
            nc.vector.tensor_scalar(C_r[b][:], P[b][:], nlr_vec[:, 0:1], None,
                                    OP.mult)

        wta = [w0T for _ in range(NB)]       # wn_t^T     (streamed, step t)
        wta_dt = [BF16] * NB
        y_cur = [None] * NB                  # wn produced at end of step t-1

        def emit_early_mms(b, t):
            """id-matmul (-lr*p fold) + previous-iterate matmuls; operands
            ready a step early, so these run during the previous step's
            elementwise."""
            pw = ps_w.tile([128, N], F32, tag="psW", name="psW")
            if mdt(t) == BF16:
                nc.tensor.matmul(pw[:], ID_b[:], C_b[b][:], start=True,
                                 stop=False)
            else:
                nc.tensor.matmul(pw[:], ID_r[:], C_r[b][:], start=True,
                                 stop=False)
            Amm = A05b if wta_dt[b] == BF16 else A05r
            for k in range(NK):
                nc.tensor.matmul(pw[:], wta[b][:, 128 * k:128 * (k + 1)],
                                 Amm[k][:], start=False, stop=False)
            return pw

        def emit_h2(b, t):
            """transpose+copy wn(t) -> new wta; halves pipelined."""
            dt_n = mdt(t)
            IDmm = {BF16: ID_b, F32R: ID_r, F32: ID}[dt_n]
            y = y_cur[b]
            pt = ps_t.tile([128, N], dt_n, tag="psT", name="psT")
            nwa = wtpool.tile([128, N], dt_n, tag=f"wta{b}", name=f"wta{b}")
            for k in range(NK):
                sl = slice(128 * k, 128 * (k + 1))
                nc.tensor.transpose(pt[:, sl], y[:, sl], IDmm[:])
                if k == 0:
                    nc.vector.tensor_copy(nwa[:, sl], pt[:, sl])
                else:
                    nc.scalar.copy(nwa[:, sl], pt[:, sl])
            wta[b] = nwa
            wta_dt[b] = dt_n

        def emit_late_mms(b, t, pw):
            Amm = A15b if mdt(t) == BF16 else A15r
            for k in range(NK):
                nc.tensor.matmul(pw[:], wta[b][:, 128 * k:128 * (k + 1)],
                                 Amm[k][:], start=False, stop=(k == NK - 1))

        def emit_dve_block(b, t, pw):
            """w = relu(z + th) with rowsum (ACT); ic = 1/sv; wn = w*ic."""
            w = wpool.tile([128, N], edt(t), tag=f"w{b}", name=f"w{b}")
            nc.scalar.activation(w[:], pw[:], RELU, bias=th[b],
                                 accum_out=sv[b])
            nc.vector.reciprocal(ic[b], sv[b])
            if t == n_steps - 1:
                wn = ypool.tile([128, N], F32, tag=f"y{b}", name=f"y{b}")
                nc.vector.tensor_scalar(wn[:], w[:], ic[b], None, OP.mult)
                nc.sync.dma_start(out_w[128 * b:128 * (b + 1), :], wn[:])
            else:
                wn = ypool.tile([128, N], mdt(t + 1), tag=f"y{b}", name=f"y{b}")
                nc.vector.tensor_scalar(wn[:], w[:], ic[b], None, OP.mult)
            y_cur[b] = wn
            return w

        def emit_theta(b):
            """lagged Newton update for theta (negated: th stores -theta)."""
            nc.gpsimd.tensor_scalar(dl[b], sv[b], 1.0, None, OP.subtract)
            nc.gpsimd.tensor_tensor(dl[b], dl[b], icn[b], OP.mult)
            nc.gpsimd.tensor_tensor(th[b], th[b], dl[b], OP.subtract)

        def emit_trio(b, w):
            m = mpool.tile([128, N], BF16, tag=f"m{b}", name=f"m{b}")
            nc.vector.tensor_scalar(m[:], w[:], 0.0, None,
                                    OP.is_gt, OP.add, accum_out=cv[b])
            nc.gpsimd.tensor_scalar(cc[b], cv[b], 1.0, None, OP.max)
            nc.vector.reciprocal(icn[b], cc[b])

        # ================= cold start: step 0, both chains =================
        pws = []
        for b in range(NB):
            pws.append(emit_early_mms(b, 0))
        for b in range(NB):
            emit_late_mms(b, 0, pws[b])     # wta == w0T for both groups
        # th0 = -(sum(z) - 1)/N
        for b in range(NB):
            scr = rpool.tile([128, N], BF16, tag=f"r{b}", name=f"r{b}")
            nc.vector.tensor_scalar(scr[:], pws[b][:], 0.0, 0.0, OP.add,
                                    OP.add, accum_out=sv[b])
            nc.vector.tensor_scalar(th[b], sv[b], 1.0, -1.0 / N,
                                    OP.subtract, OP.mult)
        for it in range(k0):
            for b in range(NB):
                r = rpool.tile([128, N], BF16, tag=f"r{b}", name=f"r{b}")
                nc.scalar.activation(r[:], pws[b][:], RELU, bias=th[b],
                                     accum_out=sv[b])
                emit_trio(b, r)
            for b in range(NB):
                emit_theta(b)
        for b in range(NB):
            w = emit_dve_block(b, 0, pws[b])
            emit_trio(b, w)
            emit_theta(b)
        emit_tail_builds()

        # ================= steady-state rounds =================
        def emit_chain_step(b, t):
            pw = emit_early_mms(b, t)
            emit_h2(b, t)
            emit_late_mms(b, t, pw)
            return pw

        for t in range(1, n_steps + 1):
            items = []
            if t >= 2:
                items.append((1, t - 1))
            if t < n_steps:
                items.append((0, t))
            pw_map = {}
            for b, tt in items:
                pw_map[b] = emit_chain_step(b, tt)
            for b, tt in items:
                w = emit_dve_block(b, tt, pw_map[b])
                if tt < n_steps - 1:
                    if tt % CNT_EVERY == 0:
                        emit_trio(b, w)
                    emit_theta(b)


def build_nc(**kw):
    nc = bacc.Bacc("TRN2", target_bir_lowering=False, debug=False,
                   enable_asserts=False)
    p_in = nc.dram_tensor("p", [B_CORE, N], F32, kind="ExternalInput")
    s_in = nc.dram_tensor("sigma", [N, N], F32, kind="ExternalInput")
    w_out = nc.dram_tensor("w", [B_CORE, N], F32, kind="ExternalOutput")
    with tile.TileContext(nc) as tc:
        markowitz_tile_kernel(tc, w_out.ap(), p_in.ap(), s_in.ap(), **kw)
    nc.compile()
    return nc


_NC_CACHE = {}


def kernel(p_batch: np.ndarray, Sigma: np.ndarray, **kw) -> np.ndarray:
    B = p_batch.shape[0]
    rows = B // N_CORES
    assert rows == B_CORE and Sigma.shape == (N, N)
    key = tuple(sorted(kw.items()))
    if key not in _NC_CACHE:
        _NC_CACHE[key] = build_nc(**kw)
    nc = _NC_CACHE[key]
    p32 = np.ascontiguousarray(p_batch, dtype=np.float32)
    s32 = np.ascontiguousarray(Sigma, dtype=np.float32)
    in_maps = [{"p": p32[i * rows:(i + 1) * rows], "sigma": s32}
               for i in range(N_CORES)]
    res = run_bass_kernel_spmd(nc, in_maps, core_ids=list(range(N_CORES)))
    out = np.concatenate([r["w"] for r in res.results], axis=0)
    return out.astype(p_batch.dtype, copy=False)


# revision 17
# speedup vs baseline: 1.1265x; 1.1265x over previous
"""Trainium2 Bass kernel for batched differentiable-Markowitz layer.

Solves, for each of 2048 rows p:  min_w 0.5 w'Sigma w + p'w  s.t. w in simplex,
matching a 200-step FISTA reference.  The fixed point is independent of lr and
the momentum schedule; we run 11 accelerated steps (9 bf16 + 2 float32r) with
constant momentum beta=0.5 and a serial loop engineered to be short:

  * momentum folded into the matmul: constant matrices A15 = 1.5(I - lr*Sigma)
    and A05n = -0.5(I - lr*Sigma) are pre-built, and PSUM accumulates
    z = wn_t@A15 + wn_{t-1}@A05n - lr*p in one group (the wn_{t-1} matmuls and
    the -lr*p identity-matmul run off the critical path since their operands
    exist a step early).
  * the projection threshold theta is LAGGED (updated on GpSimd off the
    critical path) and the iterate is re-normalized to the simplex sum every
    step (w_n = w/sum(w)), which stabilizes the lagged-theta iteration.  The
    whole on-path elementwise block runs in-order on DVE with no cross-engine
    hops: w = relu(z + th) with free row-sum accumulator, ic = 1/sv,
    wn = w*ic.
  * wn is transposed on the PE into the next step's stationary operand; the
    transpose/copy is split in 128-column halves so the k=0 matmul starts
    after half the copy.  Active-set count for theta's Newton step is
    refreshed every 4th step on GpSimd.
  * lr comes from a 4-iter on-device power iteration with a 1.10 safety
    factor.

Two batch chains of 128 rows run software-skewed (chain 1 one step behind).

Sharding: data-parallel over the batch, 256 rows per core, Sigma replicated,
no collectives.
"""

from contextlib import ExitStack

import numpy as np

import concourse.bass as bass  # noqa: F401
import concourse.tile as tile
from concourse import bacc, mybir
from concourse.bass_utils import run_bass_kernel_spmd

F32 = mybir.dt.float32
F32R = mybir.dt.float32r
BF16 = mybir.dt.bfloat16
OP = mybir.AluOpType
RELU = mybir.ActivationFunctionType.Relu

N = 256           # problem dimension
B_CORE = 256      # batch rows per core
N_CORES = 8
NB = B_CORE // 128
NK = N // 128

N_BF = 9          # bf16 matmul steps
N_R = 2           # float32r tail steps
K0_NEWTON = 2     # cold-start Newton iterations (step 0)
POW_ITERS = 4
L_SAFETY = 1.10
CNT_EVERY = 4     # refresh lagged 1/cnt every k-th step


def _make_identity(nc, ap, fill=1.0, base=0):
    nc.gpsimd.memset(ap, 0.0)
    nc.gpsimd.affine_select(
        out=ap, in_=ap, compare_op=OP.not_equal, fill=fill, base=base,
        pattern=[[-1, ap.shape[1]]], channel_multiplier=1)


def markowitz_tile_kernel(tc, out_w, in_p, in_sig, *,
                          n_bf=N_BF, n_r=N_R, k0=K0_NEWTON,
                          pow_iters=POW_ITERS, safety=L_SAFETY):
    nc = tc.nc
    ctx = ExitStack()
    n_steps = n_bf + n_r

    def mdt(t):          # matmul dtype of the iterate streamed at step t
        return BF16 if t < n_bf else F32R

    def edt(t):          # elementwise dtype of w at step t
        return BF16 if t < n_bf - 1 else F32

    const = ctx.enter_context(tc.tile_pool(name="const", bufs=1))
    rpool = ctx.enter_context(tc.tile_pool(name="r", bufs=4))
    wpool = ctx.enter_context(tc.tile_pool(name="w", bufs=4))
    ypool = ctx.enter_context(tc.tile_pool(name="y", bufs=4))
    wtpool = ctx.enter_context(tc.tile_pool(name="wt", bufs=6))
    mpool = ctx.enter_context(tc.tile_pool(name="m", bufs=2))
    xtpool = ctx.enter_context(tc.tile_pool(name="xt", bufs=4))
    ps_w = ctx.enter_context(tc.tile_pool(name="psw", bufs=3, space="PSUM"))
    ps_t = ctx.enter_context(tc.tile_pool(name="pst", bufs=2, space="PSUM"))
    ps_m = ctx.enter_context(tc.tile_pool(name="psm", bufs=2, space="PSUM"))

    with ctx:
        # ---- persistent state ----
        S = [const.tile([128, N], F32, name=f"S{k}") for k in range(NK)]
        P = [const.tile([128, N], F32, name=f"P{b}") for b in range(NB)]
        A15b = [const.tile([128, N], BF16, name=f"a15b{k}") for k in range(NK)]
        A05b = [const.tile([128, N], BF16, name=f"a05b{k}") for k in range(NK)]
        A15r = [const.tile([128, N], F32R, name=f"a15r{k}") for k in range(NK)]
        A05r = [const.tile([128, N], F32R, name=f"a05r{k}") for k in range(NK)]
        C_b = [const.tile([128, N], BF16, name=f"Cb{b}") for b in range(NB)]
        C_r = [const.tile([128, N], F32R, name=f"Cr{b}") for b in range(NB)]
        IA15 = [const.tile([128, N], F32, name=f"IA15{k}") for k in range(NK)]
        IA05 = [const.tile([128, N], F32, name=f"IA05{k}") for k in range(NK)]
        ID = const.tile([128, 128], F32, name="ID")
        ID_r = const.tile([128, 128], F32R, name="IDr")
        ID_b = const.tile([128, 128], BF16, name="IDb")
        ONES = const.tile([128, 1], F32, name="ONES")
        ZB = const.tile([128, N], BF16, name="ZB")
        Z1 = const.tile([128, 1], F32, name="Z1")
        w0T = const.tile([128, N], BF16, name="w0T")
        th = [const.tile([128, 1], F32, name=f"th{b}")[:] for b in range(NB)]
        sv = [const.tile([128, 1], F32, name=f"sv{b}")[:] for b in range(NB)]
        cv = [const.tile([128, 1], F32, name=f"cv{b}")[:] for b in range(NB)]
        cc = [const.tile([128, 1], F32, name=f"cc{b}")[:] for b in range(NB)]
        icn = [const.tile([128, 1], F32, name=f"icn{b}")[:] for b in range(NB)]
        ic = [const.tile([128, 1], F32, name=f"ic{b}")[:] for b in range(NB)]
        dl = [const.tile([128, 1], F32, name=f"dl{b}")[:] for b in range(NB)]
        lr_vec = const.tile([128, 1], F32, name="lrv")
        nlr_vec = const.tile([128, 1], F32, name="nlrv")
        nlr15 = const.tile([128, 1], F32, name="nlr15")
        lr05 = const.tile([128, 1], F32, name="lr05")
        ray = const.tile([1, 64], F32, name="ray")
        ray_i = const.tile([1, 64], F32, name="rayi")
        lmax = const.tile([1, 1], F32, name="lmax")
        lsafe = const.tile([1, 1], F32, name="lsafe")
        lr_s = const.tile([1, 1], F32, name="lrs")

        # ---- load inputs ----
        for k in range(NK):
            nc.sync.dma_start(S[k][:], in_sig[128 * k:128 * (k + 1), :])
        for b in range(NB):
            nc.sync.dma_start(P[b][:], in_p[128 * b:128 * (b + 1), :])

        # ---- constants ----
        _make_identity(nc, ID[:])
        nc.vector.tensor_copy(ID_r[:], ID[:])
        nc.vector.tensor_copy(ID_b[:], ID[:])
        for k in range(NK):
            _make_identity(nc, IA15[k][:], fill=1.5, base=128 * k)
            _make_identity(nc, IA05[k][:], fill=-0.5, base=128 * k)
        nc.gpsimd.memset(ONES[:], 1.0)
        nc.gpsimd.memset(ZB[:], 0.0)
        nc.gpsimd.memset(Z1[:], 0.0)
        nc.gpsimd.memset(w0T[:], 1.0 / N)

        # ---- power iteration for L (bf16, transposed layout) ----
        S_b = [const.tile([128, N], BF16, name=f"Sb{k}") for k in range(NK)]
        for k in range(NK):
            nc.vector.tensor_copy(S_b[k][:], S[k][:])
        xc = [S_b[k][:, 0:64] for k in range(NK)]
        xp = None
        for it in range(pow_iters):
            xn = []
            for j in range(NK):
                px = ps_m.tile([128, 64], F32, tag="pps", name="pps")
                for k in range(NK):
                    nc.tensor.matmul(px[:], S_b[k][:, 128 * j:128 * (j + 1)],
                                     xc[k],
                                     start=(k == 0), stop=(k == NK - 1))
                xs = xtpool.tile([128, 64], BF16, tag="xs", name="xs")
                nc.scalar.copy(xs[:], px[:])
                xn.append(xs)
            xp, xc = xc, [t[:] for t in xn]
        pnum = ps_m.tile([1, 64], F32, tag="pps", name="pps")
        pden = ps_m.tile([1, 64], F32, tag="pps", name="pps")
        for k in range(NK):
            prod_n = xtpool.tile([128, 64], F32, tag="prodn", name="prodn")
            prod_d = xtpool.tile([128, 64], F32, tag="prodd", name="prodd")
            nc.vector.tensor_tensor(prod_n[:], xc[k], xc[k], OP.mult)
            nc.vector.tensor_tensor(prod_d[:], xp[k], xc[k], OP.mult)
            nc.tensor.matmul(pnum[:], ONES[:], prod_n[:],
                             start=(k == 0), stop=(k == NK - 1))
            nc.tensor.matmul(pden[:], ONES[:], prod_d[:],
                             start=(k == 0), stop=(k == NK - 1))
        nc.vector.reciprocal(ray_i[:], pden[:])
        nc.vector.tensor_tensor(ray[:], pnum[:], ray_i[:], OP.mult)
        nc.vector.tensor_reduce(lmax[:], ray[:], axis=mybir.AxisListType.X, op=OP.max)
        nc.vector.tensor_scalar(lsafe[:], lmax[:], float(safety), None, OP.mult)
        nc.vector.reciprocal(lr_s[:], lsafe[:])
        nc.gpsimd.partition_broadcast(lr_vec[:], lr_s[:])
        nc.vector.tensor_scalar(nlr_vec[:], lr_vec[:], -1.0, None, OP.mult)
        nc.vector.tensor_scalar(nlr15[:], lr_vec[:], -1.5, None, OP.mult)
        nc.vector.tensor_scalar(lr05[:], lr_vec[:], 0.5, None, OP.mult)

        # ---- A15 = 1.5I - 1.5lr*S ; A05n = -0.5I + 0.5lr*S ; C = -lr*p ----
        for k in range(NK):
            nc.vector.scalar_tensor_tensor(A15b[k][:], S[k][:], nlr15[:, 0:1],
                                           IA15[k][:], op0=OP.mult, op1=OP.add)
            nc.vector.scalar_tensor_tensor(A05b[k][:], S[k][:], lr05[:, 0:1],
                                           IA05[k][:], op0=OP.mult, op1=OP.add)

        def emit_tail_builds():
            # f32r tail matrices; emitted after cold start so the DVE work
            # fills gaps during the first bf16 rounds.
            for k in range(NK):
                nc.vector.scalar_tensor_tensor(A15r[k][:], S[k][:],
                                               nlr15[:, 0:1], IA15[k][:],
                                               op0=OP.mult, op1=OP.add)
                nc.vector.scalar_tensor_tensor(A05r[k][:], S[k][:],
                                               lr05[:, 0:1], IA05[k][:],
                                               op0=OP.mult, op1=OP.add)
        for b in range(NB):
            nc.vector.tensor_scalar(C_b[b][:], P[b][:], nlr_vec[:, 0:1], None,
                                    OP.mult)
            nc.vector.tensor_scalar(C_r[b][:], P[b][:], nlr_vec[:, 0:1], None,
                                    OP.mult)

        wta = [w0T for _ in range(NB)]       # wn_t^T     (streamed, step t)
        wta_dt = [BF16] * NB
        y_cur = [None] * NB                  # wn produced at end of step t-1

        def emit_early_mms(b, t):
            """id-matmul (-lr*p fold) + previous-iterate matmuls; operands
            ready a step early, so these run during the previous step's
            elementwise."""
            pw = ps_w.tile([128, N], F32, tag="psW", name="psW")
            if mdt(t) == BF16:
                nc.tensor.matmul(pw[:], ID_b[:], C_b[b][:], start=True,
                                 stop=False)
            else:
                nc.tensor.matmul(pw[:], ID_r[:], C_r[b][:], start=True,
                                 stop=False)
            Amm = A05b if wta_dt[b] == BF16 else A05r
            for k in range(NK):
                nc.tensor.matmul(pw[:], wta[b][:, 128 * k:128 * (k + 1)],
                                 Amm[k][:], start=False, stop=False)
            return pw

        def emit_h2(b, t):
            """transpose+copy wn(t) -> new wta; halves pipelined."""
            dt_n = mdt(t)
            IDmm = {BF16: ID_b, F32R: ID_r, F32: ID}[dt_n]
            y = y_cur[b]
            pt = ps_t.tile([128, N], dt_n, tag="psT", name="psT")
            nwa = wtpool.tile([128, N], dt_n, tag=f"wta{b}", name=f"wta{b}")
            for k in range(NK):
                sl = slice(128 * k, 128 * (k + 1))
                nc.tensor.transpose(pt[:, sl], y[:, sl], IDmm[:])
                if k == 0:
                    nc.vector.tensor_copy(nwa[:, sl], pt[:, sl])
                else:
                    nc.scalar.copy(nwa[:, sl], pt[:, sl])
            wta[b] = nwa
            wta_dt[b] = dt_n

        def emit_late_mms(b, t, pw):
            Amm = A15b if mdt(t) == BF16 else A15r
            for k in range(NK):
                nc.tensor.matmul(pw[:], wta[b][:, 128 * k:128 * (k + 1)],
                                 Amm[k][:], start=False, stop=(k == NK - 1))

        def emit_dve_block(b, t, pw):
            """w = relu(z + th) with rowsum (ACT); ic = 1/sv; wn = w*ic."""
            w = wpool.tile([128, N], edt(t), tag=f"w{b}", name=f"w{b}")
            nc.scalar.activation(w[:], pw[:], RELU, bias=th[b],
                                 accum_out=sv[b])
            nc.vector.reciprocal(ic[b], sv[b])
            if t == n_steps - 1:
                wn = ypool.tile([128, N], F32, tag=f"y{b}", name=f"y{b}")
                nc.vector.tensor_scalar(wn[:], w[:], ic[b], None, OP.mult)
                nc.sync.dma_start(out_w[128 * b:128 * (b + 1), :], wn[:])
            else:
                wn = ypool.tile([128, N], mdt(t + 1), tag=f"y{b}", name=f"y{b}")
                nc.vector.tensor_scalar(wn[:], w[:], ic[b], None, OP.mult)
            y_cur[b] = wn
            return w

        def emit_theta(b):
            """lagged Newton update for theta (negated: th stores -theta)."""
            nc.gpsimd.tensor_scalar(dl[b], sv[b], 1.0, None, OP.subtract)
            nc.gpsimd.tensor_tensor(dl[b], dl[b], icn[b], OP.mult)
            nc.gpsimd.tensor_tensor(th[b], th[b], dl[b], OP.subtract)

        def emit_trio(b, w):
            m = mpool.tile([128, N], BF16, tag=f"m{b}", name=f"m{b}")
            nc.vector.tensor_scalar(m[:], w[:], 0.0, None,
                                    OP.is_gt, OP.add, accum_out=cv[b])
            nc.gpsimd.tensor_scalar(cc[b], cv[b], 1.0, None, OP.max)
            nc.vector.reciprocal(icn[b], cc[b])

        # ================= cold start: step 0, both chains =================
        pws = []
        for b in range(NB):
            pws.append(emit_early_mms(b, 0))
        for b in range(NB):
            emit_late_mms(b, 0, pws[b])     # wta == w0T for both groups
        # th0 = -(sum(z) - 1)/N
        for b in range(NB):
            scr = rpool.tile([128, N], BF16, tag=f"r{b}", name=f"r{b}")
            nc.vector.tensor_scalar(scr[:], pws[b][:], 0.0, 0.0, OP.add,
                                    OP.add, accum_out=sv[b])
            nc.vector.tensor_scalar(th[b], sv[b], 1.0, -1.0 / N,
                                    OP.subtract, OP.mult)
        for it in range(k0):
            for b in range(NB):
                r = rpool.tile([128, N], BF16, tag=f"r{b}", name=f"r{b}")
                nc.scalar.activation(r[:], pws[b][:], RELU, bias=th[b],
                                     accum_out=sv[b])
                emit_trio(b, r)
            for b in range(NB):
                emit_theta(b)
        for b in range(NB):
            w = emit_dve_block(b, 0, pws[b])
            emit_trio(b, w)
            emit_theta(b)
        emit_tail_builds()

        # ================= steady-state rounds =================
        def emit_chain_step(b, t):
            pw = emit_early_mms(b, t)
            emit_h2(b, t)
            emit_late_mms(b, t, pw)
            return pw

        for t in range(1, n_steps + 1):
            items = []
            if t >= 2:
                items.append((1, t - 1))
            if t < n_steps:
                items.append((0, t))
            pw_map = {}
            for b, tt in items:
                pw_map[b] = emit_chain_step(b, tt)
            for b, tt in items:
                w = emit_dve_block(b, tt, pw_map[b])
                if tt < n_steps - 1:
                    if tt % CNT_EVERY == 0:
                        emit_trio(b, w)
                    emit_theta(b)


def build_nc(**kw):
    nc = bacc.Bacc("TRN2", target_bir_lowering=False, debug=False,
                   enable_asserts=False)
    p_in = nc.dram_tensor("p", [B_CORE, N], F32, kind="ExternalInput")
    s_in = nc.dram_tensor("sigma", [N, N], F32, kind="ExternalInput")
    w_out = nc.dram_tensor("w", [B_CORE, N], F32, kind="ExternalOutput")
    with tile.TileContext(nc) as tc:
        markowitz_tile_kernel(tc, w_out.ap(), p_in.ap(), s_in.ap(), **kw)
    nc.compile()
    return nc


_NC_CACHE = {}


def kernel(p_batch: np.ndarray, Sigma: np.ndarray, **kw) -> np.ndarray:
    B = p_batch.shape[0]
    rows = B // N_CORES
    assert rows == B_CORE and Sigma.shape == (N, N)
    key = tuple(sorted(kw.items()))
    if key not in _NC_CACHE:
        _NC_CACHE[key] = build_nc(**kw)
    nc = _NC_CACHE[key]
    p32 = np.ascontiguousarray(p_batch, dtype=np.float32)
    s32 = np.ascontiguousarray(Sigma, dtype=np.float32)
    in_maps = [{"p": p32[i * rows:(i + 1) * rows], "sigma": s32}
               for i in range(N_CORES)]
    res = run_bass_kernel_spmd(nc, in_maps, core_ids=list(range(N_CORES)))
    out = np.concatenate([r["w"] for r in res.results], axis=0)
    return out.astype(p_batch.dtype, copy=False)


# revision 18
# speedup vs baseline: 1.2118x; 1.0758x over previous
"""Trainium2 Bass kernel for batched differentiable-Markowitz layer.

Solves, for each of 2048 rows p:  min_w 0.5 w'Sigma w + p'w  s.t. w in simplex,
matching a 200-step FISTA reference.  The fixed point is independent of lr and
the momentum schedule; we run 11 accelerated steps (9 bf16 + 2 float32r) with
constant momentum beta=0.5 and a serial loop engineered to be short:

  * momentum folded into the matmul: constant matrices A15 = 1.5(I - lr*Sigma)
    and A05n = -0.5(I - lr*Sigma) are pre-built, and PSUM accumulates
    z = wn_t@A15 + wn_{t-1}@A05n - lr*p in one group (the wn_{t-1} matmuls and
    the -lr*p identity-matmul run off the critical path since their operands
    exist a step early).
  * the projection threshold theta is LAGGED (updated on GpSimd off the
    critical path) and the iterate is re-normalized to the simplex sum every
    step (w_n = w/sum(w)), which stabilizes the lagged-theta iteration.  The
    whole on-path elementwise block runs in-order on DVE with no cross-engine
    hops: w = relu(z + th) with free row-sum accumulator, ic = 1/sv,
    wn = w*ic.
  * wn is transposed on the PE into the next step's stationary operand; the
    transpose/copy is split in 128-column halves so the k=0 matmul starts
    after half the copy.  Active-set count for theta's Newton step is
    refreshed every 4th step on GpSimd.
  * lr comes from a 4-iter on-device power iteration with a 1.10 safety
    factor.

Two batch chains of 128 rows run software-skewed (chain 1 one step behind).

Sharding: data-parallel over the batch, 256 rows per core, Sigma replicated,
no collectives.
"""

from contextlib import ExitStack

import numpy as np

import concourse.bass as bass  # noqa: F401
import concourse.tile as tile
from concourse import bacc, mybir
from concourse.bass_utils import run_bass_kernel_spmd

F32 = mybir.dt.float32
F32R = mybir.dt.float32r
BF16 = mybir.dt.bfloat16
OP = mybir.AluOpType
RELU = mybir.ActivationFunctionType.Relu

N = 256           # problem dimension
B_CORE = 256      # batch rows per core
N_CORES = 8
NB = B_CORE // 128
NK = N // 128

N_BF = 9          # bf16 matmul steps
N_R = 2           # float32r tail steps
K0_NEWTON = 2     # cold-start Newton iterations (step 0)
POW_ITERS = 4
L_SAFETY = 1.10
CNT_EVERY = 4     # refresh lagged 1/cnt every k-th step


def _make_identity(nc, ap, fill=1.0, base=0):
    nc.gpsimd.memset(ap, 0.0)
    nc.gpsimd.affine_select(
        out=ap, in_=ap, compare_op=OP.not_equal, fill=fill, base=base,
        pattern=[[-1, ap.shape[1]]], channel_multiplier=1)


def markowitz_tile_kernel(tc, out_w, in_p, in_sig, *,
                          n_bf=N_BF, n_r=N_R, k0=K0_NEWTON,
                          pow_iters=POW_ITERS, safety=L_SAFETY):
    nc = tc.nc
    ctx = ExitStack()
    n_steps = n_bf + n_r

    def mdt(t):          # matmul dtype of the iterate streamed at step t
        return BF16 if t < n_bf else F32R

    def edt(t):          # elementwise dtype of w at step t
        return BF16 if t < n_bf - 1 else F32

    const = ctx.enter_context(tc.tile_pool(name="const", bufs=1))
    rpool = ctx.enter_context(tc.tile_pool(name="r", bufs=4))
    wpool = ctx.enter_context(tc.tile_pool(name="w", bufs=4))
    ypool = ctx.enter_context(tc.tile_pool(name="y", bufs=4))
    wtpool = ctx.enter_context(tc.tile_pool(name="wt", bufs=6))
    mpool = ctx.enter_context(tc.tile_pool(name="m", bufs=2))
    xtpool = ctx.enter_context(tc.tile_pool(name="xt", bufs=4))
    ps_w = ctx.enter_context(tc.tile_pool(name="psw", bufs=3, space="PSUM"))
    ps_t = ctx.enter_context(tc.tile_pool(name="pst", bufs=2, space="PSUM"))
    ps_m = ctx.enter_context(tc.tile_pool(name="psm", bufs=2, space="PSUM"))

    with ctx:
        # ---- persistent state ----
        S = [const.tile([128, N], F32, name=f"S{k}") for k in range(NK)]
        P = [const.tile([128, N], F32, name=f"P{b}") for b in range(NB)]
        A15b = [const.tile([128, N], BF16, name=f"a15b{k}") for k in range(NK)]
        A05b = [const.tile([128, N], BF16, name=f"a05b{k}") for k in range(NK)]
        A15r = [const.tile([128, N], F32R, name=f"a15r{k}") for k in range(NK)]
        A05r = [const.tile([128, N], F32R, name=f"a05r{k}") for k in range(NK)]
        C_b = [const.tile([128, N], BF16, name=f"Cb{b}") for b in range(NB)]
        C_r = [const.tile([128, N], F32R, name=f"Cr{b}") for b in range(NB)]
        IA15 = [const.tile([128, N], F32, name=f"IA15{k}") for k in range(NK)]
        IA05 = [const.tile([128, N], F32, name=f"IA05{k}") for k in range(NK)]
        ID = const.tile([128, 128], F32, name="ID")
        ID_r = const.tile([128, 128], F32R, name="IDr")
        ID_b = const.tile([128, 128], BF16, name="IDb")
        ONES = const.tile([128, 1], F32, name="ONES")
        ZB = const.tile([128, N], BF16, name="ZB")
        Z1 = const.tile([128, 1], F32, name="Z1")
        w0T = const.tile([128, N], BF16, name="w0T")
        th = [const.tile([128, 1], F32, name=f"th{b}")[:] for b in range(NB)]
        sv = [const.tile([128, 1], F32, name=f"sv{b}")[:] for b in range(NB)]
        cv = [const.tile([128, 1], F32, name=f"cv{b}")[:] for b in range(NB)]
        cc = [const.tile([128, 1], F32, name=f"cc{b}")[:] for b in range(NB)]
        icn = [const.tile([128, 1], F32, name=f"icn{b}")[:] for b in range(NB)]
        ic = [const.tile([128, 1], F32, name=f"ic{b}")[:] for b in range(NB)]
        dl = [const.tile([128, 1], F32, name=f"dl{b}")[:] for b in range(NB)]
        lr_vec = const.tile([128, 1], F32, name="lrv")
        nlr_vec = const.tile([128, 1], F32, name="nlrv")
        nlr15 = const.tile([128, 1], F32, name="nlr15")
        lr05 = const.tile([128, 1], F32, name="lr05")
        ray = const.tile([1, 64], F32, name="ray")
        ray_i = const.tile([1, 64], F32, name="rayi")
        lmax = const.tile([1, 1], F32, name="lmax")
        lsafe = const.tile([1, 1], F32, name="lsafe")
        lr_s = const.tile([1, 1], F32, name="lrs")

        # ---- load inputs ----
        for k in range(NK):
            nc.sync.dma_start(S[k][:], in_sig[128 * k:128 * (k + 1), :])
        for b in range(NB):
            nc.sync.dma_start(P[b][:], in_p[128 * b:128 * (b + 1), :])

        # ---- constants ----
        _make_identity(nc, ID[:])
        nc.vector.tensor_copy(ID_r[:], ID[:])
        nc.vector.tensor_copy(ID_b[:], ID[:])
        for k in range(NK):
            _make_identity(nc, IA15[k][:], fill=1.5, base=128 * k)
            _make_identity(nc, IA05[k][:], fill=-0.5, base=128 * k)
        nc.gpsimd.memset(ONES[:], 1.0)
        nc.gpsimd.memset(ZB[:], 0.0)
        nc.gpsimd.memset(Z1[:], 0.0)
        nc.gpsimd.memset(w0T[:], 1.0 / N)

        # ---- power iteration for L (bf16, transposed layout) ----
        S_b = [const.tile([128, N], BF16, name=f"Sb{k}") for k in range(NK)]
        for k in range(NK):
            nc.vector.tensor_copy(S_b[k][:], S[k][:])
        xc = [S_b[k][:, 0:64] for k in range(NK)]
        xp = None
        for it in range(pow_iters):
            xn = []
            for j in range(NK):
                px = ps_m.tile([128, 64], F32, tag="pps", name="pps")
                for k in range(NK):
                    nc.tensor.matmul(px[:], S_b[k][:, 128 * j:128 * (j + 1)],
                                     xc[k],
                                     start=(k == 0), stop=(k == NK - 1))
                xs = xtpool.tile([128, 64], BF16, tag="xs", name="xs")
                nc.scalar.copy(xs[:], px[:])
                xn.append(xs)
            xp, xc = xc, [t[:] for t in xn]
        pnum = ps_m.tile([1, 64], F32, tag="pps", name="pps")
        pden = ps_m.tile([1, 64], F32, tag="pps", name="pps")
        for k in range(NK):
            prod_n = xtpool.tile([128, 64], F32, tag="prodn", name="prodn")
            prod_d = xtpool.tile([128, 64], F32, tag="prodd", name="prodd")
            nc.vector.tensor_tensor(prod_n[:], xc[k], xc[k], OP.mult)
            nc.vector.tensor_tensor(prod_d[:], xp[k], xc[k], OP.mult)
            nc.tensor.matmul(pnum[:], ONES[:], prod_n[:],
                             start=(k == 0), stop=(k == NK - 1))
            nc.tensor.matmul(pden[:], ONES[:], prod_d[:],
                             start=(k == 0), stop=(k == NK - 1))
        nc.vector.reciprocal(ray_i[:], pden[:])
        nc.vector.tensor_tensor(ray[:], pnum[:], ray_i[:], OP.mult)
        nc.vector.tensor_reduce(lmax[:], ray[:], axis=mybir.AxisListType.X, op=OP.max)
        nc.vector.tensor_scalar(lsafe[:], lmax[:], float(safety), None, OP.mult)
        nc.vector.reciprocal(lr_s[:], lsafe[:])
        nc.gpsimd.partition_broadcast(lr_vec[:], lr_s[:])
        nc.vector.tensor_scalar(nlr_vec[:], lr_vec[:], -1.0, None, OP.mult)
        nc.vector.tensor_scalar(nlr15[:], lr_vec[:], -1.5, None, OP.mult)
        nc.vector.tensor_scalar(lr05[:], lr_vec[:], 0.5, None, OP.mult)

        # ---- A15 = 1.5I - 1.5lr*S ; A05n = -0.5I + 0.5lr*S ; C = -lr*p ----
        for k in range(NK):
            nc.vector.scalar_tensor_tensor(A15b[k][:], S[k][:], nlr15[:, 0:1],
                                           IA15[k][:], op0=OP.mult, op1=OP.add)
            nc.vector.scalar_tensor_tensor(A05b[k][:], S[k][:], lr05[:, 0:1],
                                           IA05[k][:], op0=OP.mult, op1=OP.add)

        def emit_tail_builds():
            # f32r tail matrices; emitted after cold start so the DVE work
            # fills gaps during the first bf16 rounds.
            for k in range(NK):
                nc.vector.scalar_tensor_tensor(A15r[k][:], S[k][:],
                                               nlr15[:, 0:1], IA15[k][:],
                                               op0=OP.mult, op1=OP.add)
                nc.vector.scalar_tensor_tensor(A05r[k][:], S[k][:],
                                               lr05[:, 0:1], IA05[k][:],
                                               op0=OP.mult, op1=OP.add)
        for b in range(NB):
            nc.vector.tensor_scalar(C_b[b][:], P[b][:], nlr_vec[:, 0:1], None,
                                    OP.mult)
            nc.vector.tensor_scalar(C_r[b][:], P[b][:], nlr_vec[:, 0:1], None,
                                    OP.mult)

        wta = [w0T for _ in range(NB)]       # wn_t^T     (streamed, step t)
        wta_dt = [BF16] * NB
        y_cur = [None] * NB                  # wn produced at end of step t-1

        def emit_early_mms(b, t):
            """id-matmul (-lr*p fold) + previous-iterate matmuls; operands
            ready a step early, so these run during the previous step's
            elementwise."""
            pw = ps_w.tile([128, N], F32, tag="psW", name="psW")
            if mdt(t) == BF16:
                nc.tensor.matmul(pw[:], ID_b[:], C_b[b][:], start=True,
                                 stop=False)
            else:
                nc.tensor.matmul(pw[:], ID_r[:], C_r[b][:], start=True,
                                 stop=False)
            Amm = A05b if wta_dt[b] == BF16 else A05r
            for k in range(NK):
                nc.tensor.matmul(pw[:], wta[b][:, 128 * k:128 * (k + 1)],
                                 Amm[k][:], start=False, stop=False)
            return pw

        def emit_h2(b, t):
            """transpose+copy wn(t) -> new wta; halves pipelined."""
            dt_n = mdt(t)
            IDmm = {BF16: ID_b, F32R: ID_r, F32: ID}[dt_n]
            y = y_cur[b]
            pt = ps_t.tile([128, N], dt_n, tag="psT", name="psT")
            nwa = wtpool.tile([128, N], dt_n, tag=f"wta{b}", name=f"wta{b}")
            for k in range(NK):
                sl = slice(128 * k, 128 * (k + 1))
                nc.tensor.transpose(pt[:, sl], y[:, sl], IDmm[:])
                if k == 0:
                    nc.vector.tensor_copy(nwa[:, sl], pt[:, sl])
                else:
                    nc.scalar.copy(nwa[:, sl], pt[:, sl])
            wta[b] = nwa
            wta_dt[b] = dt_n

        def emit_late_mms(b, t, pw):
            Amm = A15b if mdt(t) == BF16 else A15r
            for k in range(NK):
                nc.tensor.matmul(pw[:], wta[b][:, 128 * k:128 * (k + 1)],
                                 Amm[k][:], start=False, stop=(k == NK - 1))

        def emit_dve_block(b, t, pw):
            """w = relu(z + th) with rowsum (ACT); ic = 1/sv; wn = w*ic."""
            w = wpool.tile([128, N], edt(t), tag=f"w{b}", name=f"w{b}")
            nc.scalar.activation(w[:], pw[:], RELU, bias=th[b],
                                 accum_out=sv[b])
            nc.vector.reciprocal(ic[b], sv[b])
            if t == n_steps - 1:
                wn = ypool.tile([128, N], F32, tag=f"y{b}", name=f"y{b}")
                nc.vector.tensor_scalar(wn[:], w[:], ic[b], None, OP.mult)
                nc.sync.dma_start(out_w[128 * b:128 * (b + 1), :], wn[:])
            else:
                wn = ypool.tile([128, N], mdt(t + 1), tag=f"y{b}", name=f"y{b}")
                nc.vector.tensor_scalar(wn[:], w[:], ic[b], None, OP.mult)
            y_cur[b] = wn
            return w

        def emit_theta(b):
            """lagged Newton update for theta (negated: th stores -theta)."""
            nc.vector.tensor_scalar(dl[b], sv[b], 1.0, None, OP.subtract)
            nc.vector.tensor_tensor(dl[b], dl[b], icn[b], OP.mult)
            nc.vector.tensor_tensor(th[b], th[b], dl[b], OP.subtract)

        def emit_trio(b, w):
            m = mpool.tile([128, N], BF16, tag=f"m{b}", name=f"m{b}")
            nc.vector.tensor_scalar(m[:], w[:], 0.0, None,
                                    OP.is_gt, OP.add, accum_out=cv[b])
            nc.vector.tensor_scalar(cc[b], cv[b], 1.0, None, OP.max)
            nc.vector.reciprocal(icn[b], cc[b])

        # ================= cold start: step 0, both chains =================
        pws = []
        for b in range(NB):
            pws.append(emit_early_mms(b, 0))
        for b in range(NB):
            emit_late_mms(b, 0, pws[b])     # wta == w0T for both groups
        # th0 = -(sum(z) - 1)/N
        for b in range(NB):
            scr = rpool.tile([128, N], BF16, tag=f"r{b}", name=f"r{b}")
            nc.vector.tensor_scalar(scr[:], pws[b][:], 0.0, 0.0, OP.add,
                                    OP.add, accum_out=sv[b])
            nc.vector.tensor_scalar(th[b], sv[b], 1.0, -1.0 / N,
                                    OP.subtract, OP.mult)
        for it in range(k0):
            for b in range(NB):
                r = rpool.tile([128, N], BF16, tag=f"r{b}", name=f"r{b}")
                nc.scalar.activation(r[:], pws[b][:], RELU, bias=th[b],
                                     accum_out=sv[b])
                emit_trio(b, r)
            for b in range(NB):
                emit_theta(b)
        for b in range(NB):
            w = emit_dve_block(b, 0, pws[b])
            emit_trio(b, w)
            emit_theta(b)
        emit_tail_builds()

        # ================= steady-state rounds =================
        def emit_chain_step(b, t):
            pw = emit_early_mms(b, t)
            emit_h2(b, t)
            emit_late_mms(b, t, pw)
            return pw

        for t in range(1, n_steps + 1):
            items = []
            if t >= 2:
                items.append((1, t - 1))
            if t < n_steps:
                items.append((0, t))
            pw_map = {}
            for b, tt in items:
                pw_map[b] = emit_chain_step(b, tt)
            for b, tt in items:
                w = emit_dve_block(b, tt, pw_map[b])
                if tt < n_steps - 1:
                    if tt % CNT_EVERY == 0:
                        emit_trio(b, w)
                    emit_theta(b)


def build_nc(**kw):
    nc = bacc.Bacc("TRN2", target_bir_lowering=False, debug=False,
                   enable_asserts=False)
    p_in = nc.dram_tensor("p", [B_CORE, N], F32, kind="ExternalInput")
    s_in = nc.dram_tensor("sigma", [N, N], F32, kind="ExternalInput")
    w_out = nc.dram_tensor("w", [B_CORE, N], F32, kind="ExternalOutput")
    with tile.TileContext(nc) as tc:
        markowitz_tile_kernel(tc, w_out.ap(), p_in.ap(), s_in.ap(), **kw)
    nc.compile()
    return nc


_NC_CACHE = {}


def kernel(p_batch: np.ndarray, Sigma: np.ndarray, **kw) -> np.ndarray:
    B = p_batch.shape[0]
    rows = B // N_CORES
    assert rows == B_CORE and Sigma.shape == (N, N)
    key = tuple(sorted(kw.items()))
    if key not in _NC_CACHE:
        _NC_CACHE[key] = build_nc(**kw)
    nc = _NC_CACHE[key]
    p32 = np.ascontiguousarray(p_batch, dtype=np.float32)
    s32 = np.ascontiguousarray(Sigma, dtype=np.float32)
    in_maps = [{"p": p32[i * rows:(i + 1) * rows], "sigma": s32}
               for i in range(N_CORES)]
    res = run_bass_kernel_spmd(nc, in_maps, core_ids=list(range(N_CORES)))
    out = np.concatenate([r["w"] for r in res.results], axis=0)
    return out.astype(p_batch.dtype, copy=False)


# revision 20
# speedup vs baseline: 1.2292x; 1.0143x over previous
"""Trainium2 Bass kernel for batched differentiable-Markowitz layer.

Solves, for each of 2048 rows p:  min_w 0.5 w'Sigma w + p'w  s.t. w in simplex,
matching a 200-step FISTA reference.  The fixed point is independent of lr and
the momentum schedule; we run 11 accelerated steps (9 bf16 + 2 float32r) with
constant momentum beta=0.5 and a serial loop engineered to be short:

  * momentum folded into the matmul: constant matrices A15 = 1.5(I - lr*Sigma)
    and A05n = -0.5(I - lr*Sigma) are pre-built, and PSUM accumulates
    z = wn_t@A15 + wn_{t-1}@A05n - lr*p in one group (the wn_{t-1} matmuls and
    the -lr*p identity-matmul run off the critical path since their operands
    exist a step early).
  * the projection threshold theta is LAGGED (updated on GpSimd off the
    critical path) and the iterate is re-normalized to the simplex sum every
    step (w_n = w/sum(w)), which stabilizes the lagged-theta iteration.  The
    whole on-path elementwise block runs in-order on DVE with no cross-engine
    hops: w = relu(z + th) with free row-sum accumulator, ic = 1/sv,
    wn = w*ic.
  * wn is transposed on the PE into the next step's stationary operand; the
    transpose/copy is split in 128-column halves so the k=0 matmul starts
    after half the copy.  Active-set count for theta's Newton step is
    refreshed every 4th step on GpSimd.
  * lr comes from a 4-iter on-device power iteration with a 1.10 safety
    factor.

Two batch chains of 128 rows run software-skewed (chain 1 one step behind).

Sharding: data-parallel over the batch, 256 rows per core, Sigma replicated,
no collectives.
"""

from contextlib import ExitStack

import numpy as np

import concourse.bass as bass  # noqa: F401
import concourse.tile as tile
from concourse import bacc, mybir
from concourse.bass_utils import run_bass_kernel_spmd

F32 = mybir.dt.float32
F32R = mybir.dt.float32r
BF16 = mybir.dt.bfloat16
OP = mybir.AluOpType
RELU = mybir.ActivationFunctionType.Relu

N = 256           # problem dimension
B_CORE = 256      # batch rows per core
N_CORES = 8
NB = B_CORE // 128
NK = N // 128

N_BF = 9          # bf16 matmul steps
N_R = 2           # float32r tail steps
K0_NEWTON = 2     # cold-start Newton iterations (step 0)
POW_ITERS = 4
L_SAFETY = 1.10
CNT_EVERY = 4     # refresh lagged 1/cnt every k-th step


def _make_identity(nc, ap, fill=1.0, base=0):
    nc.gpsimd.memset(ap, 0.0)
    nc.gpsimd.affine_select(
        out=ap, in_=ap, compare_op=OP.not_equal, fill=fill, base=base,
        pattern=[[-1, ap.shape[1]]], channel_multiplier=1)


def markowitz_tile_kernel(tc, out_w, in_p, in_sig, *,
                          n_bf=N_BF, n_r=N_R, k0=K0_NEWTON,
                          pow_iters=POW_ITERS, safety=L_SAFETY):
    nc = tc.nc
    ctx = ExitStack()
    n_steps = n_bf + n_r

    def mdt(t):          # matmul dtype of the iterate streamed at step t
        return BF16 if t < n_bf else F32R

    def edt(t):          # elementwise dtype of w at step t
        return BF16 if t < n_bf - 1 else F32

    const = ctx.enter_context(tc.tile_pool(name="const", bufs=1))
    rpool = ctx.enter_context(tc.tile_pool(name="r", bufs=4))
    wpool = ctx.enter_context(tc.tile_pool(name="w", bufs=4))
    ypool = ctx.enter_context(tc.tile_pool(name="y", bufs=4))
    wtpool = ctx.enter_context(tc.tile_pool(name="wt", bufs=6))
    mpool = ctx.enter_context(tc.tile_pool(name="m", bufs=2))
    xtpool = ctx.enter_context(tc.tile_pool(name="xt", bufs=4))
    ps_w = ctx.enter_context(tc.tile_pool(name="psw", bufs=3, space="PSUM"))
    ps_t = ctx.enter_context(tc.tile_pool(name="pst", bufs=2, space="PSUM"))
    ps_m = ctx.enter_context(tc.tile_pool(name="psm", bufs=2, space="PSUM"))

    with ctx:
        # ---- persistent state ----
        S = [const.tile([128, N], F32, name=f"S{k}") for k in range(NK)]
        P = [const.tile([128, N], F32, name=f"P{b}") for b in range(NB)]
        A15b = [const.tile([128, N], BF16, name=f"a15b{k}") for k in range(NK)]
        A05b = [const.tile([128, N], BF16, name=f"a05b{k}") for k in range(NK)]
        A15r = [const.tile([128, N], F32R, name=f"a15r{k}") for k in range(NK)]
        A05r = [const.tile([128, N], F32R, name=f"a05r{k}") for k in range(NK)]
        C_b = [const.tile([128, N], BF16, name=f"Cb{b}") for b in range(NB)]
        C_r = [const.tile([128, N], F32R, name=f"Cr{b}") for b in range(NB)]
        IA15 = [const.tile([128, N], F32, name=f"IA15{k}") for k in range(NK)]
        IA05 = [const.tile([128, N], F32, name=f"IA05{k}") for k in range(NK)]
        ID = const.tile([128, 128], F32, name="ID")
        ID_r = const.tile([128, 128], F32R, name="IDr")
        ID_b = const.tile([128, 128], BF16, name="IDb")
        ONES = const.tile([128, 1], F32, name="ONES")
        ONESR = const.tile([1, 128], F32, name="ONESR")
        ZB = const.tile([128, N], BF16, name="ZB")
        Z1 = const.tile([128, 1], F32, name="Z1")
        w0T = const.tile([128, N], BF16, name="w0T")
        th = [const.tile([128, 1], F32, name=f"th{b}")[:] for b in range(NB)]
        sv = [const.tile([128, 1], F32, name=f"sv{b}")[:] for b in range(NB)]
        cv = [const.tile([128, 1], F32, name=f"cv{b}")[:] for b in range(NB)]
        cc = [const.tile([128, 1], F32, name=f"cc{b}")[:] for b in range(NB)]
        icn = [const.tile([128, 1], F32, name=f"icn{b}")[:] for b in range(NB)]
        ic = [const.tile([128, 1], F32, name=f"ic{b}")[:] for b in range(NB)]
        dl = [const.tile([128, 1], F32, name=f"dl{b}")[:] for b in range(NB)]
        lr_vec = const.tile([128, 1], F32, name="lrv")
        nlr_vec = const.tile([128, 1], F32, name="nlrv")
        nlr15 = const.tile([128, 1], F32, name="nlr15")
        lr05 = const.tile([128, 1], F32, name="lr05")
        ray = const.tile([1, 64], F32, name="ray")
        ray_i = const.tile([1, 64], F32, name="rayi")
        lmax = const.tile([1, 1], F32, name="lmax")
        lsafe = const.tile([1, 1], F32, name="lsafe")
        lr_s = const.tile([1, 1], F32, name="lrs")

        # ---- load inputs ----
        for k in range(NK):
            nc.sync.dma_start(S[k][:], in_sig[128 * k:128 * (k + 1), :])
        for b in range(NB):
            nc.sync.dma_start(P[b][:], in_p[128 * b:128 * (b + 1), :])

        # ---- constants ----
        _make_identity(nc, ID[:])
        nc.vector.tensor_copy(ID_r[:], ID[:])
        nc.vector.tensor_copy(ID_b[:], ID[:])
        for k in range(NK):
            _make_identity(nc, IA15[k][:], fill=1.5, base=128 * k)
            _make_identity(nc, IA05[k][:], fill=-0.5, base=128 * k)
        nc.gpsimd.memset(ONES[:], 1.0)
        nc.gpsimd.memset(ONESR[:], 1.0)
        nc.gpsimd.memset(ZB[:], 0.0)
        nc.gpsimd.memset(Z1[:], 0.0)
        nc.gpsimd.memset(w0T[:], 1.0 / N)

        # ---- power iteration for L (bf16, transposed layout) ----
        S_b = [const.tile([128, N], BF16, name=f"Sb{k}") for k in range(NK)]
        for k in range(NK):
            nc.vector.tensor_copy(S_b[k][:], S[k][:])
        xc = [S_b[k][:, 0:64] for k in range(NK)]
        xp = None
        for it in range(pow_iters):
            xn = []
            for j in range(NK):
                px = ps_m.tile([128, 64], F32, tag="pps", name="pps")
                for k in range(NK):
                    nc.tensor.matmul(px[:], S_b[k][:, 128 * j:128 * (j + 1)],
                                     xc[k],
                                     start=(k == 0), stop=(k == NK - 1))
                xs = xtpool.tile([128, 64], BF16, tag="xs", name="xs")
                nc.scalar.copy(xs[:], px[:])
                xn.append(xs)
            xp, xc = xc, [t[:] for t in xn]
        pnum = ps_m.tile([1, 64], F32, tag="pps", name="pps")
        pden = ps_m.tile([1, 64], F32, tag="pps", name="pps")
        for k in range(NK):
            prod_n = xtpool.tile([128, 64], F32, tag="prodn", name="prodn")
            prod_d = xtpool.tile([128, 64], F32, tag="prodd", name="prodd")
            nc.vector.tensor_tensor(prod_n[:], xc[k], xc[k], OP.mult)
            nc.vector.tensor_tensor(prod_d[:], xp[k], xc[k], OP.mult)
            nc.tensor.matmul(pnum[:], ONES[:], prod_n[:],
                             start=(k == 0), stop=(k == NK - 1))
            nc.tensor.matmul(pden[:], ONES[:], prod_d[:],
                             start=(k == 0), stop=(k == NK - 1))
        nc.vector.reciprocal(ray_i[:], pden[:])
        nc.vector.tensor_tensor(ray[:], pnum[:], ray_i[:], OP.mult)
        nc.vector.tensor_reduce(lmax[:], ray[:], axis=mybir.AxisListType.X, op=OP.max)
        nc.vector.tensor_scalar(lsafe[:], lmax[:], float(safety), None, OP.mult)
        nc.vector.reciprocal(lr_s[:], lsafe[:])
        blr = ps_m.tile([128, 1], F32, tag="pps", name="blr")
        nc.tensor.matmul(blr[:], ONESR[:], lr_s[:], start=True, stop=True)
        nc.vector.tensor_copy(lr_vec[:], blr[:])
        nc.vector.tensor_scalar(nlr_vec[:], lr_vec[:], -1.0, None, OP.mult)
        nc.vector.tensor_scalar(nlr15[:], lr_vec[:], -1.5, None, OP.mult)
        nc.vector.tensor_scalar(lr05[:], lr_vec[:], 0.5, None, OP.mult)

        # ---- A15 = 1.5I - 1.5lr*S ; A05n = -0.5I + 0.5lr*S ; C = -lr*p ----
        for k in range(NK):
            nc.vector.scalar_tensor_tensor(A15b[k][:], S[k][:], nlr15[:, 0:1],
                                           IA15[k][:], op0=OP.mult, op1=OP.add)
            nc.vector.scalar_tensor_tensor(A05b[k][:], S[k][:], lr05[:, 0:1],
                                           IA05[k][:], op0=OP.mult, op1=OP.add)

        def emit_tail_builds():
            # f32r tail matrices; emitted after cold start so the DVE work
            # fills gaps during the first bf16 rounds.
            for k in range(NK):
                nc.vector.scalar_tensor_tensor(A15r[k][:], S[k][:],
                                               nlr15[:, 0:1], IA15[k][:],
                                               op0=OP.mult, op1=OP.add)
                nc.vector.scalar_tensor_tensor(A05r[k][:], S[k][:],
                                               lr05[:, 0:1], IA05[k][:],
                                               op0=OP.mult, op1=OP.add)
        for b in range(NB):
            nc.vector.tensor_scalar(C_b[b][:], P[b][:], nlr_vec[:, 0:1], None,
                                    OP.mult)
            nc.vector.tensor_scalar(C_r[b][:], P[b][:], nlr_vec[:, 0:1], None,
                                    OP.mult)

        wta = [w0T for _ in range(NB)]       # wn_t^T     (streamed, step t)
        wta_dt = [BF16] * NB
        y_cur = [None] * NB                  # wn produced at end of step t-1

        def emit_early_mms(b, t):
            """id-matmul (-lr*p fold) + previous-iterate matmuls; operands
            ready a step early, so these run during the previous step's
            elementwise."""
            pw = ps_w.tile([128, N], F32, tag="psW", name="psW")
            if mdt(t) == BF16:
                nc.tensor.matmul(pw[:], ID_b[:], C_b[b][:], start=True,
                                 stop=False)
            else:
                nc.tensor.matmul(pw[:], ID_r[:], C_r[b][:], start=True,
                                 stop=False)
            Amm = A05b if wta_dt[b] == BF16 else A05r
            for k in range(NK):
                nc.tensor.matmul(pw[:], wta[b][:, 128 * k:128 * (k + 1)],
                                 Amm[k][:], start=False, stop=False)
            return pw

        def emit_h2(b, t):
            """transpose+copy wn(t) -> new wta; halves pipelined."""
            dt_n = mdt(t)
            IDmm = {BF16: ID_b, F32R: ID_r, F32: ID}[dt_n]
            y = y_cur[b]
            pt = ps_t.tile([128, N], dt_n, tag="psT", name="psT")
            nwa = wtpool.tile([128, N], dt_n, tag=f"wta{b}", name=f"wta{b}")
            for k in range(NK):
                sl = slice(128 * k, 128 * (k + 1))
                nc.tensor.transpose(pt[:, sl], y[:, sl], IDmm[:])
                if k == 0:
                    nc.vector.tensor_copy(nwa[:, sl], pt[:, sl])
                else:
                    nc.scalar.copy(nwa[:, sl], pt[:, sl])
            wta[b] = nwa
            wta_dt[b] = dt_n

        def emit_late_mms(b, t, pw):
            Amm = A15b if mdt(t) == BF16 else A15r
            for k in range(NK):
                nc.tensor.matmul(pw[:], wta[b][:, 128 * k:128 * (k + 1)],
                                 Amm[k][:], start=False, stop=(k == NK - 1))

        def emit_dve_block(b, t, pw):
            """w = relu(z + th) with rowsum (ACT); ic = 1/sv; wn = w*ic."""
            w = wpool.tile([128, N], edt(t), tag=f"w{b}", name=f"w{b}")
            nc.scalar.activation(w[:], pw[:], RELU, bias=th[b],
                                 accum_out=sv[b])
            nc.vector.reciprocal(ic[b], sv[b])
            if t == n_steps - 1:
                wn = ypool.tile([128, N], F32, tag=f"y{b}", name=f"y{b}")
                nc.vector.tensor_scalar(wn[:], w[:], ic[b], None, OP.mult)
                nc.sync.dma_start(out_w[128 * b:128 * (b + 1), :], wn[:])
            else:
                wn = ypool.tile([128, N], mdt(t + 1), tag=f"y{b}", name=f"y{b}")
                nc.vector.tensor_scalar(wn[:], w[:], ic[b], None, OP.mult)
            y_cur[b] = wn
            return w

        def emit_theta(b):
            """lagged Newton update for theta (negated: th stores -theta)."""
            nc.vector.tensor_scalar(dl[b], sv[b], 1.0, None, OP.subtract)
            nc.vector.tensor_tensor(dl[b], dl[b], icn[b], OP.mult)
            nc.vector.tensor_tensor(th[b], th[b], dl[b], OP.subtract)

        def emit_trio(b, w):
            m = mpool.tile([128, N], BF16, tag=f"m{b}", name=f"m{b}")
            nc.vector.tensor_scalar(m[:], w[:], 0.0, None,
                                    OP.is_gt, OP.add, accum_out=cv[b])
            nc.vector.tensor_scalar(cc[b], cv[b], 1.0, None, OP.max)
            nc.vector.reciprocal(icn[b], cc[b])

        # ================= cold start: step 0, both chains =================
        pws = []
        for b in range(NB):
            pws.append(emit_early_mms(b, 0))
        for b in range(NB):
            emit_late_mms(b, 0, pws[b])     # wta == w0T for both groups
        # th0 = -(sum(z) - 1)/N
        for b in range(NB):
            scr = rpool.tile([128, N], BF16, tag=f"r{b}", name=f"r{b}")
            nc.vector.tensor_scalar(scr[:], pws[b][:], 0.0, 0.0, OP.add,
                                    OP.add, accum_out=sv[b])
            nc.vector.tensor_scalar(th[b], sv[b], 1.0, -1.0 / N,
                                    OP.subtract, OP.mult)
        for it in range(k0):
            for b in range(NB):
                r = rpool.tile([128, N], BF16, tag=f"r{b}", name=f"r{b}")
                nc.scalar.activation(r[:], pws[b][:], RELU, bias=th[b],
                                     accum_out=sv[b])
                emit_trio(b, r)
            for b in range(NB):
                emit_theta(b)
        for b in range(NB):
            w = emit_dve_block(b, 0, pws[b])
            emit_trio(b, w)
            emit_theta(b)
        emit_tail_builds()

        # ================= steady-state rounds =================
        def emit_chain_step(b, t):
            pw = emit_early_mms(b, t)
            emit_h2(b, t)
            emit_late_mms(b, t, pw)
            return pw

        for t in range(1, n_steps + 1):
            items = []
            if t >= 2:
                items.append((1, t - 1))
            if t < n_steps:
                items.append((0, t))
            pw_map = {}
            for b, tt in items:
                pw_map[b] = emit_chain_step(b, tt)
            for b, tt in items:
                w = emit_dve_block(b, tt, pw_map[b])
                if tt < n_steps - 1:
                    if tt % CNT_EVERY == 0:
                        emit_trio(b, w)
                    emit_theta(b)


def build_nc(**kw):
    nc = bacc.Bacc("TRN2", target_bir_lowering=False, debug=False,
                   enable_asserts=False)
    p_in = nc.dram_tensor("p", [B_CORE, N], F32, kind="ExternalInput")
    s_in = nc.dram_tensor("sigma", [N, N], F32, kind="ExternalInput")
    w_out = nc.dram_tensor("w", [B_CORE, N], F32, kind="ExternalOutput")
    with tile.TileContext(nc) as tc:
        markowitz_tile_kernel(tc, w_out.ap(), p_in.ap(), s_in.ap(), **kw)
    nc.compile()
    return nc


_NC_CACHE = {}


def kernel(p_batch: np.ndarray, Sigma: np.ndarray, **kw) -> np.ndarray:
    B = p_batch.shape[0]
    rows = B // N_CORES
    assert rows == B_CORE and Sigma.shape == (N, N)
    key = tuple(sorted(kw.items()))
    if key not in _NC_CACHE:
        _NC_CACHE[key] = build_nc(**kw)
    nc = _NC_CACHE[key]
    p32 = np.ascontiguousarray(p_batch, dtype=np.float32)
    s32 = np.ascontiguousarray(Sigma, dtype=np.float32)
    in_maps = [{"p": p32[i * rows:(i + 1) * rows], "sigma": s32}
               for i in range(N_CORES)]
    res = run_bass_kernel_spmd(nc, in_maps, core_ids=list(range(N_CORES)))
    out = np.concatenate([r["w"] for r in res.results], axis=0)
    return out.astype(p_batch.dtype, copy=False)
